# revision 1
# baseline (speedup 1.0000x reference)
"""Causal multi-head self-attention (with RoPE, V-uses-Q-projection bug preserved)
as a Bass/Tile kernel for 8 Trainium2 NeuronCores.

Sharding: core c -> batch b = c//4, head-group g = c%4 (4 heads of 16).
Each core computes its 4 heads' attention for its batch and a partial
output projection (contraction over its 256 features of Wo); partials are
summed per batch on the host (the tensor-parallel all-reduce of the O
projection, done during unsharding).

Device math (per core):
  xT[Dm, T] (stationary) x wqk[Dm, 512] -> Q|K [T, 256] each (+bias)
  V = pre-RoPE Q (the reference's V-uses-Wq bug)
  RoPE on Q, K in a per-head permuted feature layout (evens first, odds
  second) so rotate-half becomes contiguous 32-blocks.
  PE-transpose roped Q,K -> QT,KT [256, T].
  Per head: S^T[k,q] = KT_h^T-tile @ QT_h (contraction over d_k=64),
  causal mask added on diagonal 128-blocks, exp(S/8) via ACT into SBUF,
  attn@V as out2T[65, q] = V'^T @ expST accumulated over k-tiles, where
  V' = [V_h | 1] so row 64 is the softmax denominator.  Divide rows 0..63
  by row 64, then O-projection partial = headsT^T @ WoT (+bo on group-0
  cores only).

All matmuls run as float32r (full PE rate, ~1e-3 mean rel err).
"""

import os
from contextlib import ExitStack

import numpy as np

import concourse.bass as bass
import concourse.mybir as mybir
import concourse.tile as tile
from concourse import bacc
from concourse.bass import ds, ts
from concourse.masks import make_identity

F32 = mybir.dt.float32
F32R = mybir.dt.float32r
AF = mybir.ActivationFunctionType
ALU = mybir.AluOpType

B, T, D, H, DK = 2, 2048, 1024, 16, 64
THETA = 10000.0
NCORE, GPB = 8, 4          # cores; head-groups per batch
HPC = H // GPB             # heads per core = 4
OC = HPC * DK              # per-core projected features = 256
NT = T // 128              # 16 t-tiles
NDC = D // 128             # 8 contraction chunks
NEG = -1.0e30


def build_kernel(dbg=False, reps=1):
    nc = bacc.Bacc("TRN2", target_bir_lowering=False, debug=False)

    xT = nc.dram_tensor("xT", [D, T], F32R, kind="ExternalInput")
    wqk = nc.dram_tensor("wqk", [D, 2 * OC], F32R, kind="ExternalInput")
    wo = nc.dram_tensor("wo", [OC, D], F32R, kind="ExternalInput")
    bqk = nc.dram_tensor("bqk", [1, 2 * OC], F32, kind="ExternalInput")
    bo = nc.dram_tensor("bo", [1, D], F32, kind="ExternalInput")
    pos = nc.dram_tensor("pos", [T], mybir.dt.int32, kind="ExternalInput")
    invf = nc.dram_tensor("invf", [1, DK // 2], F32, kind="ExternalInput")
    maskd = nc.dram_tensor("maskd", [128, 128], F32, kind="ExternalInput")
    out = nc.dram_tensor("out", [T, D], F32, kind="ExternalOutput")

    with tile.TileContext(nc) as tc:
      for _rep in range(reps):
       with ExitStack() as top:
            # ---- long-lived pools ----
            consts = top.enter_context(tc.tile_pool(name="consts", bufs=1))
            wpool = top.enter_context(tc.tile_pool(name="weights", bufs=1))
            vk = top.enter_context(tc.tile_pool(name="vk", bufs=1))
            qtkt = top.enter_context(tc.tile_pool(name="qtkt", bufs=1))
            heads_pool = top.enter_context(tc.tile_pool(name="heads", bufs=1))

            # ---- constants / tables ----
            ident = consts.tile([128, 128], F32)
            make_identity(nc, ident[:])
            maskd_sb = consts.tile([128, 128], F32)
            nc.sync.dma_start(maskd_sb[:], maskd.ap())
            bqk_rep = consts.tile([128, 2 * OC], F32)
            nc.sync.dma_start(bqk_rep[:], bqk.ap().to_broadcast((128, 2 * OC)))
            bo_rep = consts.tile([128, D], F32)
            nc.sync.dma_start(bo_rep[:], bo.ap().to_broadcast((128, D)))
            invf_rep = consts.tile([128, DK // 2], F32)
            nc.sync.dma_start(invf_rep[:], invf.ap().to_broadcast((128, DK // 2)))

            posi = consts.tile([128, NT], mybir.dt.int32)
            nc.sync.dma_start(posi[:], pos.ap().rearrange("(j p) -> p j", p=128))
            posf = consts.tile([128, NT], F32)
            nc.vector.tensor_copy(posf[:], posi[:])

            # angle/sin/cos tables: ang[p, j, i] = pos[128j+p] * invf[i], range-
            # reduced into [-pi, pi] via Cody-Waite (k = round(ang/2pi)).
            NF = NT * (DK // 2)  # 512 flat table cols
            ang = consts.tile([128, NF], F32)
            for j in range(NT):
                nc.vector.tensor_scalar_mul(
                    ang[:, ts(j, DK // 2)], invf_rep[:], posf[:, j : j + 1]
                )
            TWO_PI = 2.0 * np.pi
            # split 2*pi into three ~11-bit chunks for the Cody-Waite cascade
            c1 = float(np.float32(6.28125))
            c2 = float(np.float32(0.0019531250))  # 2^-9
            c3 = float(np.float32(TWO_PI - 6.28125 - 0.0019531250))
            kidx = consts.tile([128, NF], mybir.dt.int32)
            nc.vector.tensor_scalar(
                kidx[:], ang[:], float(1.0 / TWO_PI), 0.5, ALU.mult, ALU.add
            )
            kf = consts.tile([128, NF], F32)
            nc.vector.tensor_copy(kf[:], kidx[:])
            angr = consts.tile([128, NF], F32)
            nc.vector.cody_waite_cascade(angr[:], ang[:], kf[:], c1, c2, c3)
            # k can be off by one (f32->int rounding differs between HW and sim);
            # wrap both branches back into [-pi, pi] before the Sin spline.
            angs = consts.tile([128, NF], F32)
            nc.vector.add_range_wrap(angs[:], angr[:], 0.0, float(np.pi), TWO_PI)
            sin32 = consts.tile([128, NT, DK // 2], F32)
            nc.scalar.activation(
                sin32[:].rearrange("p a b -> p (a b)"), angs[:], AF.Sin
            )
            angc = consts.tile([128, NF], F32)
            nc.vector.add_range_wrap(
                angc[:], angr[:], float(np.pi / 2), float(np.pi), TWO_PI
            )
            cosF = consts.tile([128, NT, DK], F32)  # [cos | cos]
            angc_v = angc[:].rearrange("p (a b) -> p a b", a=NT)
            nc.scalar.activation(cosF[:, :, 0 : DK // 2], angc_v, AF.Sin)
            nc.scalar.activation(cosF[:, :, DK // 2 : DK], angc_v, AF.Sin)

            # bias row + ones row so O-proj adds bo inside the matmul (K=1)
            bo_r = consts.tile([1, D], F32R)
            nc.sync.dma_start(bo_r[:], bo.ap().bitcast(F32R))
            ones_o = consts.tile([1, 128], F32R)
            nc.vector.memset(ones_o[:].bitcast(F32), 1.0)

            # ---- weights (wo only; wqk is phase-scoped) ----
            wo_sb = [wpool.tile([128, D], F32R, tag=f"wo{jc}", name=f"wo_sb{jc}") for jc in range(2)]
            for jc in range(2):
                nc.sync.dma_start(wo_sb[jc][:], wo.ap()[ts(jc, 128), :])

            # ---- V (=Q+bias, pre-rope): per head [64 feats | 64 ones], so the
            # attn@V' matmul emits [heads(64) ; denom-replicated(64)] rows.
            # (walrus requires a single-free-dim stationary AP, so the ones block
            # is duplicated per head.) ----
            v_sb = vk.tile([128, NT, HPC, 2 * DK], F32R)
            nc.gpsimd.memset(v_sb[:, :, :, DK:].bitcast(F32), 1.0)

            qt_sb = [qtkt.tile([128, T // 2], F32R, tag=f"qt{i}", name=f"qt_sb{i}") for i in range(4)]
            kt_sb = [qtkt.tile([128, T // 2], F32R, tag=f"kt{i}", name=f"kt_sb{i}") for i in range(4)]

            NG = 4      # t-groups
            GT = NT // NG  # t-tiles per group = 4

            def rope(eng, src_view, out_tile, pool, tg):
                """One t-group of rotary embedding on engine `eng`.
                src_view/out views are [p, GT, HPC, DK]."""
                m = pool.tile([128, GT, OC], F32, tag="rope_m", name=f"rm{tg}")
                s = pool.tile([128, GT, HPC, DK // 2], F32, tag="rope_s", name=f"rs{tg}")
                x = src_view
                x1 = x[:, :, :, 0 : DK // 2]
                x2 = x[:, :, :, DK // 2 : DK]
                tsl = slice(tg * GT, (tg + 1) * GT)
                cos_bc = cosF[:, tsl].unsqueeze(2).to_broadcast((128, GT, HPC, DK))
                sin_bc = sin32[:, tsl].unsqueeze(2).to_broadcast((128, GT, HPC, DK // 2))
                mv = m[:].rearrange("p t (h f) -> p t h f", h=HPC)
                rv = out_tile
                eng.tensor_tensor(mv, x, cos_bc, ALU.mult)
                eng.tensor_tensor(s[:], x2, sin_bc, ALU.mult)
                eng.tensor_tensor(
                    rv[:, :, :, 0 : DK // 2], mv[:, :, :, 0 : DK // 2], s[:], ALU.subtract
                )
                eng.tensor_tensor(s[:], x1, sin_bc, ALU.mult)
                eng.tensor_tensor(
                    rv[:, :, :, DK // 2 : DK], mv[:, :, :, DK // 2 : DK], s[:], ALU.add
                )

            # ---- phase B+C+D per t-group: projection, rope, transpose ----
            with tc.tile_pool(name="xt", bufs=2) as xtp, \
                 tc.tile_pool(name="rope", bufs=2) as ropep, \
                 tc.tile_pool(name="proj_w", bufs=1) as pwp, \
                 tc.tile_pool(name="ps_proj", bufs=int(os.environ.get("PSP_BUFS", "6")), space="PSUM") as psp, \
                 tc.tile_pool(name="ps_tp", bufs=int(os.environ.get("TP_BUFS", "2")), space="PSUM") as pst:
                wqk_sb = pwp.tile([128, NDC, 2 * OC], F32R)
                for dc in range(NDC):
                    nc.sync.dma_start(wqk_sb[:, dc, :], wqk.ap()[ts(dc, 128), :])
                for tg in range(NG):
                    k_sb = ropep.tile([128, GT, OC], F32, tag="k_sb", name=f"ks{tg}")
                    xt = xtp.tile([128, NDC, 512], F32R, tag="xt")
                    for dc in range(NDC):
                        nc.sync.dma_start(
                            xt[:, dc, :], xT.ap()[ts(dc, 128), ts(tg, 512)]
                        )
                    for tl in range(GT):
                        t = GT * tg + tl
                        ps = psp.tile([128, 2 * OC], F32, tag="pproj")
                        for dc in range(NDC):
                            nc.tensor.matmul(
                                ps[:],
                                xt[:, dc, ts(tl, 128)],
                                wqk_sb[:, dc, :],
                                start=(dc == 0),
                                stop=(dc == NDC - 1),
                            )
                        # V part (blocks 0..3) and K part
                        vv = v_sb[:, t, :, 0:DK]
                        nc.vector.tensor_tensor(
                            vv,
                            ps[:, 0:OC].rearrange("p (h f) -> p h f", h=HPC),
                            bqk_rep[:, 0:OC].rearrange("p (h f) -> p h f", h=HPC),
                            ALU.add,
                        )
                        nc.vector.tensor_add(
                            k_sb[:, tl, :], ps[:, OC : 2 * OC], bqk_rep[:, OC : 2 * OC]
                        )
                    # rope this t-group: Q on DVE (from v_sb), K on GpSimd
                    q_rope = ropep.tile([128, GT, HPC, DK], F32, tag="q_rope", name=f"qr{tg}")
                    v_view = v_sb[:, ts(tg, GT), :, 0:DK].bitcast(F32)
                    rope(nc.vector, v_view, q_rope[:], ropep, tg)
                    k_rope = ropep.tile([128, GT, HPC, DK], F32, tag="k_rope", name=f"kr{tg}")
                    k_view = k_sb[:].rearrange("p t (h f) -> p t h f", h=HPC)
                    rope(nc.gpsimd, k_view, k_rope[:], ropep, tg)
                    # transposes: 4 t-tiles per oc into one [128, 512] psum
                    for srcv, dst in ((q_rope, qt_sb), (k_rope, kt_sb)):
                        sv = srcv[:].rearrange("p t h f -> p t (h f)")
                        for oc in range(2):
                            tp = pst.tile([128, 512], F32, tag="tp")
                            for tl in range(GT):
                                nc.tensor.transpose(
                                    tp[:, ts(tl, 128)], sv[:, tl, ts(oc, 128)], ident[:]
                                )
                            nc.vector.tensor_copy(
                                dst[oc * 2 + tg // 2][:, ts(tg % 2, 512)], tp[:]
                            )

            # ---- phase E: attention, q-chunk (1024) outer, head inner ----
            heads_t = [
                heads_pool.tile([128, T // 2], F32R, tag=f"ht{i}", name=f"heads_t{i}")
                for i in range(4)  # index = jc*2 + c2
            ]
            scale = float(1.0 / np.sqrt(DK))
            CW = 1024  # q-chunk width

            def attention_chunk(c2, pssc, expp, divp, pso2, heads=range(HPC)):
                    q0 = CW * c2
                    for h in heads:
                        oc, ro = h // 2, 64 * (h % 2)
                        qt_h = qt_sb[oc * 2 + c2][ds(ro, 64), :]
                        o2 = pso2.tile([128, CW], F32, tag="o2")
                        nkt = 8 * (c2 + 1)
                        for kt in range(nkt):
                            qs = max(q0, 128 * kt)
                            cw = q0 + CW - qs  # exp/scores width for this kt
                            et = expp.tile([128, CW], F32R, tag="et")  # cols = q - qs
                            sc = pssc.tile([128, CW], F32, tag="sc")
                            for n5 in range((cw + 511) // 512):
                                ns = qs + 512 * n5
                                nw = min(512, q0 + CW - ns)
                                nc.tensor.matmul(
                                    sc[:, ds(512 * n5, nw)],
                                    kt_sb[oc * 2 + kt // 8][ds(ro, 64), ts(kt % 8, 128)],
                                    qt_h[:, ds(ns - q0, nw)],
                                    start=True,
                                    stop=True,
                                )
                            if qs == 128 * kt:  # diagonal block lives in this chunk
                                nc.vector.tensor_add(
                                    sc[:, 0:128], sc[:, 0:128], maskd_sb[:]
                                )
                            nc.scalar.activation(
                                et[:, ds(0, cw)], sc[:, ds(0, cw)], AF.Exp, scale=scale
                            )
                            if dbg and c2 == 0 and h == 0 and kt == 0:
                                d_et = divp.tile([128, CW], F32, name="dbg_et", bufs=1)
                                nc.vector.tensor_copy(d_et[:], et[:].bitcast(F32))
                                dt_ = nc.dram_tensor("d_et", [128, CW], F32, kind="ExternalOutput")
                                nc.sync.dma_start(dt_.ap(), d_et[:])
                            # attn @ V' accumulation over kt into [65, CW]
                            for c in (2 * c2, 2 * c2 + 1):
                                ce = 512 * (c + 1)
                                if ce <= qs:
                                    continue
                                ns = max(qs, 512 * c)
                                nw = ce - ns
                                nc.tensor.matmul(
                                    o2[:, ds(ns - q0, nw)],
                                    v_sb[:, kt, h, :],
                                    et[:, ds(ns - qs, nw)],
                                    start=(kt == 0),
                                    stop=(kt == min(4 * c + 3, nkt - 1)),
                                )
                        # rows 64..127 of o2 all hold the softmax denominator.
                        # Copy o2 out first so its PSUM banks free for the next
                        # head, then reciprocal, DMA-shift down to partitions
                        # 0..63, multiply rows 0..63, ship to heads_t.
                        oc2 = divp.tile([128, CW], F32, tag="oc2")
                        nc.vector.tensor_copy(oc2[:], o2[:])
                        rec_t = divp.tile([128, CW], F32, tag="rec_t")
                        nc.vector.reciprocal(rec_t[ds(DK, DK), :], oc2[ds(DK, DK), :])
                        rec_lo = divp.tile([64, CW], F32, tag="rec_lo")
                        nc.sync.dma_start(rec_lo[:], rec_t[ds(DK, DK), :])
                        stage = divp.tile([64, CW], F32R, tag="stage")
                        nc.vector.tensor_tensor(stage[:], oc2[0:DK, :], rec_lo[:], ALU.mult)
                        nc.sync.dma_start(
                            heads_t[oc * 2 + c2][ds(ro, 64), :], stage[:]
                        )
                        if dbg and c2 == 0 and h == 0:
                            d_o2 = divp.tile([128, CW], F32, name="dbg_o2", bufs=1)
                            nc.vector.tensor_copy(d_o2[:], o2[:])
                            d_rec = divp.tile([64, CW], F32, name="dbg_rec", bufs=1)
                            nc.vector.tensor_copy(d_rec[:], rec_lo[:])
                            for nm, ap in (("d_o2", d_o2[:]), ("d_rec", d_rec[:])):
                                dt_ = nc.dram_tensor(nm, list(ap.shape), F32, kind="ExternalOutput")
                                nc.sync.dma_start(dt_.ap(), ap)

            def oproj(t, psop, outp):
                    c2 = t // 8
                    for ic in range(2):
                        po = psop.tile([128, 512], F32, tag="po")
                        for jc in range(2):
                            nc.tensor.matmul(
                                po[:],
                                heads_t[jc * 2 + c2][:, ts(t - 8 * c2, 128)],
                                wo_sb[jc][:, ts(ic, 512)],
                                start=(jc == 0),
                                stop=False,
                            )
                        nc.tensor.matmul(
                            po[:], ones_o[:], bo_r[:, ts(ic, 512)],
                            start=False, stop=True,
                        )
                        ot = outp.tile([128, 512], F32, tag="ot")
                        if (2 * t + ic) % 2 == 0:
                            nc.vector.tensor_copy(ot[:], po[:])
                        else:
                            nc.scalar.copy(ot[:], po[:])
                        nc.sync.dma_start(out.ap()[ts(t, 128), ts(ic, 512)], ot[:])

            # ---- phases E+F: attention, then output projection ----
            with tc.tile_pool(name="expst", bufs=int(os.environ.get("ET_BUFS", "6"))) as expp, \
                 tc.tile_pool(name="divp", bufs=int(os.environ.get("DIV_BUFS", "2"))) as divp, \
                 tc.tile_pool(name="ps_sc", bufs=int(os.environ.get("SC_BUFS", "3")), space="PSUM") as pssc, \
                 tc.tile_pool(name="ps_o2", bufs=1, space="PSUM") as pso2:
                attention_chunk(0, pssc, expp, divp, pso2)
                attention_chunk(1, pssc, expp, divp, pso2)
            with tc.tile_pool(name="outp", bufs=int(os.environ.get("OUTP_BUFS", "6")) ) as outp, \
                 tc.tile_pool(name="ps_op", bufs=int(os.environ.get("OP_BUFS", "6")), space="PSUM") as psop:
                for t in range(NT):
                    oproj(t, psop, outp)

            if dbg:
                dbg_outs = {
                    "d_cos": cosF[:].rearrange("p a b -> p (a b)"),
                    "d_sin": sin32[:].rearrange("p a b -> p (a b)"),
                    "d_v": v_sb[:].bitcast(F32).rearrange("p a b c -> p (a b c)"),
                    "d_qt0": qt_sb[0][:].bitcast(F32),
                    "d_kt0": kt_sb[0][:].bitcast(F32),
                    "d_qt1": qt_sb[2][:].bitcast(F32),
                    "d_ht0": heads_t[0][:].bitcast(F32),
                    "d_ht1": heads_t[1][:].bitcast(F32),
                }
                for name, ap in dbg_outs.items():
                    dt_ = nc.dram_tensor(name, list(ap.shape), F32, kind="ExternalOutput")
                    nc.sync.dma_start(dt_.ap(), ap)

    nc.compile()
    return nc


_NC_CACHE = None


def _get_nc():
    global _NC_CACHE
    if _NC_CACHE is None:
        _NC_CACHE = build_kernel()
    return _NC_CACHE


_PERM = np.concatenate([np.arange(0, DK, 2), np.arange(1, DK, 2)])


def make_in_maps(in_features, token_positions, Wq, bq, Wk, bk, Wo, bo):
    x = np.ascontiguousarray(np.asarray(in_features, dtype=np.float32))
    pos = np.ascontiguousarray(np.asarray(token_positions, dtype=np.int32))
    Wq = np.asarray(Wq, dtype=np.float32)
    bq = np.asarray(bq, dtype=np.float32)
    Wk = np.asarray(Wk, dtype=np.float32)
    bk = np.asarray(bk, dtype=np.float32)
    Wo = np.asarray(Wo, dtype=np.float32)
    bo = np.asarray(bo, dtype=np.float32)
    invf = (1.0 / THETA ** (np.arange(0, DK, 2, dtype=np.float32) / DK)).astype(
        np.float32
    )[None, :]
    ii = np.arange(128)
    maskd = np.where(ii[None, :] >= ii[:, None], 0.0, NEG).astype(np.float32)
    in_maps = []
    for c in range(NCORE):
        b, g = c // GPB, c % GPB
        cols = np.concatenate([DK * (HPC * g + hh) + _PERM for hh in range(HPC)])
        in_maps.append(
            {
                "xT": np.ascontiguousarray(x[b].T),
                "wqk": np.ascontiguousarray(
                    np.concatenate([Wq[cols].T, Wk[cols].T], axis=1)
                ),
                "wo": np.ascontiguousarray(Wo[:, cols].T),
                "bqk": np.ascontiguousarray(
                    np.concatenate([bq[cols], bk[cols]])[None, :]
                ),
                "bo": np.ascontiguousarray(
                    (bo if g == 0 else np.zeros_like(bo))[None, :]
                ),
                "pos": pos,
                "invf": invf,
                "maskd": maskd,
            }
        )
    return in_maps


def kernel(in_features, token_positions, Wq, bq, Wk, bk, Wv=None, bv=None, Wo=None, bo=None):
    from concourse import bass_utils

    nc = _get_nc()
    in_maps = make_in_maps(in_features, token_positions, Wq, bq, Wk, bk, Wo, bo)
    res = bass_utils.run_bass_kernel_spmd(
        nc,
        in_maps,
        core_ids=list(range(NCORE)),
        trace=bool(int(os.environ.get("KERNEL_TRACE", "0"))),
    )
    outs = [res.results[c]["out"] for c in range(NCORE)]
    full = np.stack(
        [np.sum(outs[b * GPB : (b + 1) * GPB], axis=0) for b in range(B)]
    ).astype(np.float32)
    kernel.last_results = res
    return full

